# revision 25
# baseline (speedup 1.0000x reference)
"""Trainium2 Bass kernel for additive (Bahdanau) attention.

    c[b] = softmax_t( v_a . tanh(s[b] @ W_a + h[b] @ U_a) ) @ h[b]

Shapes (hardcoded): s [32,1024] f32, h [32,2048,1024] f32,
W_a [1024,512], U_a [1024,512], v_a [512]  ->  c [32,1024] f32.

Sharding: data-parallel over batch; 8 NeuronCores x 4 batches each.
W_a/U_a/v_a replicated. No cross-core communication.

Key structural constraints learned from profiling:
  - copy-mode DMAs and xbar transpose DMAs serialize on HW (fully additive,
    measured), and the xbar runs at only ~150 GB/s -> do the transposes on
    the TensorEngine instead (is_transpose matmuls, bf16 PSUM out, DVE 2x
    copy-back) and keep the DMA pipe copy-only at HBM line rate.
  - The PE queue is in-order; any instruction waiting on a cross-engine dep
    stalls everything behind it -> defer the softmax/stage-6 tail by one
    supertile and issue v-dot matmuls only after all mains of a supertile.

Per-core pipeline, per 512-row "supertile" of h[b]:
  1. SWDGE DMA loads h f32->bf16 natural layout [128t, 4ts, 1024d].
  2. TensorE transposes 32 [128,128] chunks (identity moving operand) into
     bf16 PSUM banks; VectorE copies them to SBUF as [128 d_lo, dc, ts, t].
  3. TensorE: 32 bf16 matmuls (U_a chunks stationary) -> PSUM E [a_chunk, t].
  4. ScalarE: tanh(E + bias) with per-partition bias (W_a @ s), bf16 out.
  5. TensorE: v-dot (v chunk stationary, E moving) -> logit row [1, 512].
  6. ScalarE: exp (+ accumulated row sum S) -> p row (unnormalized softmax;
     e is bounded by |v|_1 so no max subtraction is needed).
  7. TensorE: transpose p chunks to [128,1] via K=1 matmul vs [1,1] ones.
  8. TensorE: c += p^T @ h_natural (PSUM-accumulated over the whole batch).
  9. batch end: c = c * (1/S) on VectorE; all output DMAs at kernel end.
"""

import numpy as np

import concourse.bacc as bacc
import concourse.tile as tile
import concourse.mybir as mybir
from concourse.bass_utils import run_bass_kernel_spmd

N_CORES = 8
B, T, DH, DS, A = 32, 2048, 1024, 1024, 512
BPC = B // N_CORES          # batches per core
ST = 512                    # supertile rows (t)
NST = T // ST               # supertiles per batch
NTS = ST // 128             # 128-row chunks per supertile
NDC = DH // 128             # d chunks
NAC = A // 128              # a chunks
NTCH = T // 128             # 128-row chunks per batch

F32 = mybir.dt.float32
BF16 = mybir.dt.bfloat16
AF = mybir.ActivationFunctionType


def build_nc():
    nc = bacc.Bacc("TRN2", target_bir_lowering=False, debug=False,
                   num_devices=N_CORES)
    s = nc.dram_tensor("s", [BPC, DS], F32, kind="ExternalInput").ap()
    h = nc.dram_tensor("h", [BPC, T, DH], F32, kind="ExternalInput").ap()
    W_a = nc.dram_tensor("W_a", [DS, A], F32, kind="ExternalInput").ap()
    U_a = nc.dram_tensor("U_a", [DH, A], F32, kind="ExternalInput").ap()
    v_a = nc.dram_tensor("v_a", [A], F32, kind="ExternalInput").ap()
    c = nc.dram_tensor("c", [BPC, DH], F32, kind="ExternalOutput").ap()

    with tile.TileContext(nc) as tc:
        with (
            tc.tile_pool(name="const", bufs=1) as const,
            tc.tile_pool(name="hpool", bufs=7) as hpool,
            tc.tile_pool(name="htpool", bufs=3) as htpool,
            tc.tile_pool(name="esbp", bufs=5) as esbp,
            tc.tile_pool(name="smalls", bufs=3) as smalls,
            tc.tile_pool(name="cresp", bufs=4) as cresp,
            tc.tile_pool(name="epool", bufs=2, space="PSUM") as epool,
            tc.tile_pool(name="prowp", bufs=1, space="PSUM") as prowp,
            tc.tile_pool(name="ptpp", bufs=1, space="PSUM") as ptpp,
            tc.tile_pool(name="cpool", bufs=1, space="PSUM") as cpool,
            tc.tile_pool(name="tpsp", bufs=2, space="PSUM") as tpsp,
        ):
            h_tiles = {}
            ht_tiles = {}

            def load_h(b, st):
                t = hpool.tile([128, NTS, DH], BF16, name=f"h_sb{b}_{st}",
                               tag="h_sb")
                nc.gpsimd.dma_start(
                    out=t,
                    in_=h[b, ST * st:ST * (st + 1), :]
                    .rearrange("(ts p) d -> p ts d", p=128))
                h_tiles[(b, st)] = t

            def xbar_h(b, st):
                # PE-based transpose: 32 [128,128] chunks -> 4 bf16 PSUM banks
                # (2 d-chunks each) -> DVE 2x copy to SBUF.
                # hT layout: [128 d_lo, NDC, NTS, 128 t].
                h_sb = h_tiles[(b, st)]
                ht = htpool.tile([128, NDC, NTS, 128], BF16,
                                 name=f"hT_sb{b}_{st}", tag="hT_sb")
                for dcp in range(NDC // 2):
                    tps = tpsp.tile([128, 1024], BF16,
                                    name=f"tps{b}_{st}_{dcp}", tag="tps")
                    for dch in range(2):
                        dc = 2 * dcp + dch
                        for ts in range(NTS):
                            nc.tensor.transpose(
                                tps[:, dch * 512 + ts * 128:
                                    dch * 512 + ts * 128 + 128],
                                h_sb[:, ts, 128 * dc:128 * (dc + 1)],
                                ident)
                    nc.vector.tensor_copy(
                        ht[:, 2 * dcp, :, :], tps[:, 0:512])
                    nc.vector.tensor_copy(
                        ht[:, 2 * dcp + 1, :, :], tps[:, 512:1024])
                ht_tiles[(b, st)] = ht

            from concourse.masks import make_identity
            ident = const.tile([128, 128], BF16, name="ident")
            make_identity(nc, ident)

            # -- startup: first loads+transpose ahead of consts --
            load_h(0, 0)
            xbar_h(0, 0)

            # ---- constants (copy-mode phase) ----
            U_sb = const.tile([128, NDC, A], BF16)
            nc.gpsimd.dma_start(out=U_sb, in_=U_a.rearrange("(dc p) a -> p dc a", p=128))
            load_h(0, 1)
            W_sb = const.tile([128, NDC, A], F32)
            nc.gpsimd.dma_start(out=W_sb, in_=W_a.rearrange("(dc p) a -> p dc a", p=128))
            sT_sb = const.tile([128, NDC, BPC], F32)
            for dc in range(NDC):
                nc.gpsimd.dma_start(
                    out=sT_sb[:, dc, :],
                    in_=s[:, 128 * dc:128 * (dc + 1)].rearrange("b p -> p b"))
            v_sb = const.tile([128, NAC], BF16)
            nc.gpsimd.dma_start(out=v_sb, in_=v_a.rearrange("(ac p) -> p ac", p=128))
            one1 = const.tile([1, 1], BF16)
            nc.vector.memset(one1, 1.0)
            load_h(0, 2)

            # bias[a, b] = (W_a^T s[b])[a]  stored [128 a_lo, NAC, BPC] f32
            bias_sb = const.tile([128, NAC, BPC], F32)

            def emit_bias():
                for ac in range(NAC):
                    ws_ps = epool.tile([128, BPC], F32, name=f"ws_ps{ac}",
                                       tag="e_ps")
                    for dc in range(NDC):
                        nc.tensor.matmul(ws_ps,
                                         lhsT=W_sb[:, dc, 128 * ac:128 * (ac + 1)],
                                         rhs=sT_sb[:, dc, :],
                                         start=(dc == 0), stop=(dc == NDC - 1))
                    nc.vector.tensor_copy(bias_sb[:, ac, :], ws_ps)

            def stage6a(b, st, p_exp):
                # p-row -> column transpose matmuls + copy to SBUF
                pT_ps = ptpp.tile([128, NTS], F32, name=f"pT_ps{b}_{st}",
                                  tag="pT_ps")
                for ts in range(NTS):
                    nc.tensor.matmul(pT_ps[:, ts:ts + 1],
                                     lhsT=p_exp[:, 128 * ts:128 * (ts + 1)],
                                     rhs=one1, start=True, stop=True,
                                     skip_group_check=True)
                pT_sb = smalls.tile([128, NTS], BF16, name=f"pT_sb{b}_{st}",
                                    tag="pT_sb")
                nc.vector.tensor_copy(pT_sb, pT_ps)
                return pT_sb

            def stage6b(b, st, pT_sb, c_lo, c_hi):
                # c matmuls are M=1: pack the 4 t-chunks into 4 column groups
                # (tile_position) so they run concurrently; partial sums land
                # on partitions 0/32/64/96 and are combined at batch end.
                h_sb = h_tiles.pop((b, st))
                first, last = st == 0, st == NST - 1
                for ts in range(NTS):
                    nc.tensor.matmul(c_lo[32 * ts:32 * ts + 1, :],
                                     lhsT=pT_sb[:, ts:ts + 1],
                                     rhs=h_sb[:, ts, 0:512],
                                     start=first, stop=last,
                                     tile_position=(0, 32 * ts),
                                     skip_group_check=True)
                    nc.tensor.matmul(c_hi[32 * ts:32 * ts + 1, :],
                                     lhsT=pT_sb[:, ts:ts + 1],
                                     rhs=h_sb[:, ts, 512:DH],
                                     start=first, stop=last,
                                     tile_position=(0, 32 * ts),
                                     skip_group_check=True)

            def batch_epilogue(b, c_lo, c_hi, S4_sb):
                S_sb = smalls.tile([1, 1], F32, name=f"S_sb{b}", tag="S_sb")
                nc.vector.reduce_sum(S_sb, S4_sb, axis=mybir.AxisListType.X)
                rS = smalls.tile([1, 1], F32, name=f"rS{b}", tag="rS")
                nc.vector.reciprocal(rS, S_sb)
                c4_sb = cresp.tile([128, 2, 512], F32, name=f"c4_sb{b}",
                                   tag="c4_sb", bufs=2)
                nc.vector.tensor_copy(c4_sb[:, 0, :], c_lo)
                nc.vector.tensor_copy(c4_sb[:, 1, :], c_hi)
                acc = cresp.tile([1, DH], F32, name=f"acc{b}", tag=f"acc{b}",
                                 bufs=1)
                # fold rows 0/32/64/96 with chained accumulate-add DMAs
                acc2d = acc.rearrange("o (k d) -> o k d", k=2)
                nc.gpsimd.dma_start(out=acc2d, in_=c4_sb[0:1])
                for j in range(1, NTS):
                    nc.gpsimd.dma_start(out=acc2d, in_=c4_sb[32 * j:32 * j + 1],
                                        accum_op=mybir.AluOpType.add)
                c_sb = cresp.tile([1, DH], F32, name=f"c_sb{b}", tag=f"c_sb{b}",
                                  bufs=1)
                nc.vector.tensor_scalar_mul(c_sb, acc, rS)
                return c_sb

            # ---- main loop ----
            c_out_tiles = []
            S4_tiles = {}
            pendings = []   # [b, st, p_exp, c_lo, c_hi, pT_sb] awaiting stage6
            for b in range(BPC):
                c_lo = cpool.tile([128, 512], F32, name=f"c_lo{b}", tag="c_lo")
                c_hi = cpool.tile([128, 512], F32, name=f"c_hi{b}", tag="c_hi")
                S4_sb = smalls.tile([1, NST], F32, name=f"S4_sb{b}", tag="S4_sb")
                S4_tiles[b] = S4_sb
                for st in range(NST):
                    hT_sb = ht_tiles.pop((b, st))
                    p_row = prowp.tile([1, ST], F32, name=f"p_row{b}_{st}",
                                       tag="p_row")
                    e_sbs = []
                    for ac in range(NAC):
                        e_ps = epool.tile([128, ST], F32, name=f"e_ps{b}_{st}_{ac}",
                                          tag="e_ps")
                        for dc in range(NDC):
                            nc.tensor.matmul(
                                e_ps,
                                lhsT=U_sb[:, dc, 128 * ac:128 * (ac + 1)],
                                rhs=hT_sb[:, dc, :, :],
                                start=(dc == 0), stop=(dc == NDC - 1))
                        if b == 0 and st == 0 and ac == 0:
                            emit_bias()
                        e_sb = esbp.tile([128, ST], BF16, name=f"e_sb{b}_{st}_{ac}",
                                         tag="e_sb")
                        nc.scalar.activation(e_sb, e_ps, AF.Tanh,
                                             bias=bias_sb[:, ac, b:b + 1])
                        e_sbs.append(e_sb)
                        if ac == 1 and pendings:
                            # pT matmuls of the previous supertile: their exp
                            # dep is long done; DVE copy overlaps mains ac1-3.
                            e = pendings[-1]
                            if e[5] is None:
                                e[5] = stage6a(*e[:3])
                        if ac == 2:
                            # rolling prefetch: load 2 supertiles ahead
                            glob = NST * b + st + 2
                            if glob < NST * BPC:
                                load_h(glob // NST, glob % NST)
                    # transpose the next supertile (PE + DVE copies); also
                    # gives the last tanh time before the v-dots need it.
                    glob = NST * b + st + 1
                    if glob < NST * BPC:
                        xbar_h(glob // NST, glob % NST)
                    # v-dots after all mains: their tanh deps are resolved by
                    # the time PE reaches them.
                    for ac in range(NAC):
                        nc.tensor.matmul(p_row, lhsT=v_sb[:, ac:ac + 1],
                                         rhs=e_sbs[ac],
                                         start=(ac == 0), stop=(ac == NAC - 1))

                    p_exp = smalls.tile([1, ST], BF16, name=f"p_exp{b}_{st}",
                                        tag="p_exp")
                    nc.scalar.activation(p_exp, p_row, AF.Exp,
                                         accum_out=S4_sb[:, st:st + 1])

                    if len(pendings) >= 1:
                        e = pendings.pop(0)
                        stage6b(e[0], e[1], e[5], e[3], e[4])
                        if e[1] == NST - 1:   # finished a batch
                            c_out_tiles.append(
                                (e[0], batch_epilogue(e[0], e[3], e[4],
                                                      S4_tiles[e[0]])))
                    pendings.append([b, st, p_exp, c_lo, c_hi, None])
            # drain remaining pendings
            for e in pendings:
                if e[5] is None:
                    e[5] = stage6a(*e[:3])
                stage6b(e[0], e[1], e[5], e[3], e[4])
                if e[1] == NST - 1:
                    c_out_tiles.append(
                        (e[0], batch_epilogue(e[0], e[3], e[4],
                                              S4_tiles[e[0]])))

            # ---- all output DMAs at the very end (single mode transition) --
            for pb, c_sb in c_out_tiles:
                nc.gpsimd.dma_start(out=c[pb:pb + 1, :], in_=c_sb)

    nc.finalize()
    return nc


_NC_CACHE = None


def kernel(s, h, W_a, U_a, v_a):
    global _NC_CACHE
    if _NC_CACHE is None:
        _NC_CACHE = build_nc()
    nc = _NC_CACHE
    s = np.ascontiguousarray(s, dtype=np.float32)
    h = np.ascontiguousarray(h, dtype=np.float32)
    W_a = np.ascontiguousarray(W_a, dtype=np.float32)
    U_a = np.ascontiguousarray(U_a, dtype=np.float32)
    v_a = np.ascontiguousarray(v_a, dtype=np.float32)
    in_maps = [
        {"s": s[i * BPC:(i + 1) * BPC], "h": h[i * BPC:(i + 1) * BPC],
         "W_a": W_a, "U_a": U_a, "v_a": v_a}
        for i in range(N_CORES)
    ]
    res = run_bass_kernel_spmd(nc, in_maps, core_ids=list(range(N_CORES)))
    return np.concatenate([res.results[i]["c"] for i in range(N_CORES)], axis=0)


# revision 26
# speedup vs baseline: 1.0422x; 1.0422x over previous
"""Trainium2 Bass kernel for additive (Bahdanau) attention.

    c[b] = softmax_t( v_a . tanh(s[b] @ W_a + h[b] @ U_a) ) @ h[b]

Shapes (hardcoded): s [32,1024] f32, h [32,2048,1024] f32,
W_a [1024,512], U_a [1024,512], v_a [512]  ->  c [32,1024] f32.

Sharding: data-parallel over batch; 8 NeuronCores x 4 batches each.
W_a/U_a/v_a replicated. No cross-core communication.

Key structural constraints learned from profiling:
  - copy-mode DMAs and xbar transpose DMAs serialize on HW (fully additive,
    measured), and the xbar runs at only ~150 GB/s -> do the transposes on
    the TensorEngine instead (is_transpose matmuls, bf16 PSUM out, DVE 2x
    copy-back) and keep the DMA pipe copy-only at HBM line rate.
  - The PE queue is in-order; any instruction waiting on a cross-engine dep
    stalls everything behind it -> defer the softmax/stage-6 tail by one
    supertile and issue v-dot matmuls only after all mains of a supertile.

Per-core pipeline, per 512-row "supertile" of h[b]:
  1. SWDGE DMA loads h f32->bf16 natural layout [128t, 4ts, 1024d].
  2. TensorE transposes 32 [128,128] chunks (identity moving operand) into
     bf16 PSUM banks; VectorE copies them to SBUF as [128 d_lo, dc, ts, t].
  3. TensorE: 32 bf16 matmuls (U_a chunks stationary) -> PSUM E [a_chunk, t].
  4. ScalarE: tanh(E + bias) with per-partition bias (W_a @ s), bf16 out.
  5. TensorE: v-dot (v chunk stationary, E moving) -> logit row [1, 512].
  6. ScalarE: exp (+ accumulated row sum S) -> p row (unnormalized softmax;
     e is bounded by |v|_1 so no max subtraction is needed).
  7. TensorE: transpose p chunks to [128,1] via K=1 matmul vs [1,1] ones.
  8. TensorE: c += p^T @ h_natural (PSUM-accumulated over the whole batch).
  9. batch end: c = c * (1/S) on VectorE; all output DMAs at kernel end.
"""

import numpy as np

import concourse.bacc as bacc
import concourse.tile as tile
import concourse.mybir as mybir
from concourse.bass_utils import run_bass_kernel_spmd

N_CORES = 8
B, T, DH, DS, A = 32, 2048, 1024, 1024, 512
BPC = B // N_CORES          # batches per core
ST = 512                    # supertile rows (t)
NST = T // ST               # supertiles per batch
NTS = ST // 128             # 128-row chunks per supertile
NDC = DH // 128             # d chunks
NAC = A // 128              # a chunks
NTCH = T // 128             # 128-row chunks per batch

F32 = mybir.dt.float32
BF16 = mybir.dt.bfloat16
AF = mybir.ActivationFunctionType


def build_nc():
    nc = bacc.Bacc("TRN2", target_bir_lowering=False, debug=False,
                   num_devices=N_CORES)
    s = nc.dram_tensor("s", [BPC, DS], F32, kind="ExternalInput").ap()
    h = nc.dram_tensor("h", [BPC, T, DH], F32, kind="ExternalInput").ap()
    W_a = nc.dram_tensor("W_a", [DS, A], F32, kind="ExternalInput").ap()
    U_a = nc.dram_tensor("U_a", [DH, A], F32, kind="ExternalInput").ap()
    v_a = nc.dram_tensor("v_a", [A], F32, kind="ExternalInput").ap()
    c = nc.dram_tensor("c", [BPC, DH], F32, kind="ExternalOutput").ap()

    with tile.TileContext(nc) as tc:
        with (
            tc.tile_pool(name="const", bufs=1) as const,
            tc.tile_pool(name="hpool", bufs=7) as hpool,
            tc.tile_pool(name="htpool", bufs=4) as htpool,
            tc.tile_pool(name="esbp", bufs=5) as esbp,
            tc.tile_pool(name="smalls", bufs=3) as smalls,
            tc.tile_pool(name="cresp", bufs=4) as cresp,
            tc.tile_pool(name="epool", bufs=2, space="PSUM") as epool,
            tc.tile_pool(name="prowp", bufs=1, space="PSUM") as prowp,
            tc.tile_pool(name="ptpp", bufs=1, space="PSUM") as ptpp,
            tc.tile_pool(name="cpool", bufs=1, space="PSUM") as cpool,
            tc.tile_pool(name="tpsp", bufs=2, space="PSUM") as tpsp,
        ):
            h_tiles = {}
            ht_tiles = {}

            def load_h(b, st):
                t = hpool.tile([128, NTS, DH], BF16, name=f"h_sb{b}_{st}",
                               tag="h_sb")
                nc.gpsimd.dma_start(
                    out=t,
                    in_=h[b, ST * st:ST * (st + 1), :]
                    .rearrange("(ts p) d -> p ts d", p=128))
                h_tiles[(b, st)] = t

            def xbar_h(b, st):
                # PE-based transpose: 32 [128,128] chunks -> 4 bf16 PSUM banks
                # (2 d-chunks each) -> DVE 2x copy to SBUF.
                # hT layout: [128 d_lo, NDC, NTS, 128 t].
                h_sb = h_tiles[(b, st)]
                ht = htpool.tile([128, NDC, NTS, 128], BF16,
                                 name=f"hT_sb{b}_{st}", tag="hT_sb")
                for dcp in range(NDC // 2):
                    tps = tpsp.tile([128, 1024], BF16,
                                    name=f"tps{b}_{st}_{dcp}", tag="tps")
                    for dch in range(2):
                        dc = 2 * dcp + dch
                        for ts in range(NTS):
                            nc.tensor.transpose(
                                tps[:, dch * 512 + ts * 128:
                                    dch * 512 + ts * 128 + 128],
                                h_sb[:, ts, 128 * dc:128 * (dc + 1)],
                                ident)
                    nc.vector.tensor_copy(
                        ht[:, 2 * dcp, :, :], tps[:, 0:512])
                    nc.vector.tensor_copy(
                        ht[:, 2 * dcp + 1, :, :], tps[:, 512:1024])
                ht_tiles[(b, st)] = ht

            from concourse.masks import make_identity
            ident = const.tile([128, 128], BF16, name="ident")
            make_identity(nc, ident)

            # -- startup: first load split into quarters so the first PE
            # transposes unblock per-chunk; then its transpose.
            t0 = hpool.tile([128, NTS, DH], BF16, name="h_sb0_0", tag="h_sb")
            for ts in range(NTS):
                nc.gpsimd.dma_start(
                    out=t0[:, ts],
                    in_=h[0, 128 * ts:128 * (ts + 1), :]
                    .rearrange("p d -> p d"))
            h_tiles[(0, 0)] = t0
            xbar_h(0, 0)

            # ---- constants (copy-mode phase) ----
            U_sb = const.tile([128, NDC, A], BF16)
            nc.gpsimd.dma_start(out=U_sb, in_=U_a.rearrange("(dc p) a -> p dc a", p=128))
            load_h(0, 1)
            W_sb = const.tile([128, NDC, A], F32)
            nc.gpsimd.dma_start(out=W_sb, in_=W_a.rearrange("(dc p) a -> p dc a", p=128))
            sT_sb = const.tile([128, NDC, BPC], F32)
            for dc in range(NDC):
                nc.gpsimd.dma_start(
                    out=sT_sb[:, dc, :],
                    in_=s[:, 128 * dc:128 * (dc + 1)].rearrange("b p -> p b"))
            v_sb = const.tile([128, NAC], BF16)
            nc.gpsimd.dma_start(out=v_sb, in_=v_a.rearrange("(ac p) -> p ac", p=128))
            one1 = const.tile([1, 1], BF16)
            nc.vector.memset(one1, 1.0)
            load_h(0, 2)

            # bias[a, b] = (W_a^T s[b])[a]  stored [128 a_lo, NAC, BPC] f32
            bias_sb = const.tile([128, NAC, BPC], F32)

            def emit_bias():
                for ac in range(NAC):
                    ws_ps = epool.tile([128, BPC], F32, name=f"ws_ps{ac}",
                                       tag="e_ps")
                    for dc in range(NDC):
                        nc.tensor.matmul(ws_ps,
                                         lhsT=W_sb[:, dc, 128 * ac:128 * (ac + 1)],
                                         rhs=sT_sb[:, dc, :],
                                         start=(dc == 0), stop=(dc == NDC - 1))
                    nc.vector.tensor_copy(bias_sb[:, ac, :], ws_ps)

            def stage6a(b, st, p_exp):
                # p-row -> column transpose matmuls + copy to SBUF
                pT_ps = ptpp.tile([128, NTS], F32, name=f"pT_ps{b}_{st}",
                                  tag="pT_ps")
                for ts in range(NTS):
                    nc.tensor.matmul(pT_ps[:, ts:ts + 1],
                                     lhsT=p_exp[:, 128 * ts:128 * (ts + 1)],
                                     rhs=one1, start=True, stop=True,
                                     skip_group_check=True)
                pT_sb = smalls.tile([128, NTS], BF16, name=f"pT_sb{b}_{st}",
                                    tag="pT_sb")
                nc.vector.tensor_copy(pT_sb, pT_ps)
                return pT_sb

            def stage6b(b, st, pT_sb, c_lo, c_hi):
                # c matmuls are M=1: pack the 4 t-chunks into 4 column groups
                # (tile_position) so they run concurrently; partial sums land
                # on partitions 0/32/64/96 and are combined at batch end.
                h_sb = h_tiles.pop((b, st))
                first, last = st == 0, st == NST - 1
                for ts in range(NTS):
                    nc.tensor.matmul(c_lo[32 * ts:32 * ts + 1, :],
                                     lhsT=pT_sb[:, ts:ts + 1],
                                     rhs=h_sb[:, ts, 0:512],
                                     start=first, stop=last,
                                     tile_position=(0, 32 * ts),
                                     skip_group_check=True)
                    nc.tensor.matmul(c_hi[32 * ts:32 * ts + 1, :],
                                     lhsT=pT_sb[:, ts:ts + 1],
                                     rhs=h_sb[:, ts, 512:DH],
                                     start=first, stop=last,
                                     tile_position=(0, 32 * ts),
                                     skip_group_check=True)

            def batch_epilogue(b, c_lo, c_hi, S4_sb):
                S_sb = smalls.tile([1, 1], F32, name=f"S_sb{b}", tag="S_sb")
                nc.vector.reduce_sum(S_sb, S4_sb, axis=mybir.AxisListType.X)
                rS = smalls.tile([1, 1], F32, name=f"rS{b}", tag="rS")
                nc.vector.reciprocal(rS, S_sb)
                c4_sb = cresp.tile([128, 2, 512], F32, name=f"c4_sb{b}",
                                   tag="c4_sb", bufs=2)
                nc.vector.tensor_copy(c4_sb[:, 0, :], c_lo)
                nc.vector.tensor_copy(c4_sb[:, 1, :], c_hi)
                acc = cresp.tile([1, DH], F32, name=f"acc{b}", tag=f"acc{b}",
                                 bufs=1)
                # fold rows 0/32/64/96 with chained accumulate-add DMAs
                acc2d = acc.rearrange("o (k d) -> o k d", k=2)
                nc.gpsimd.dma_start(out=acc2d, in_=c4_sb[0:1])
                for j in range(1, NTS):
                    nc.gpsimd.dma_start(out=acc2d, in_=c4_sb[32 * j:32 * j + 1],
                                        accum_op=mybir.AluOpType.add)
                c_sb = cresp.tile([1, DH], F32, name=f"c_sb{b}", tag=f"c_sb{b}",
                                  bufs=1)
                nc.vector.tensor_scalar_mul(c_sb, acc, rS)
                return c_sb

            # ---- main loop ----
            c_out_tiles = []
            S4_tiles = {}
            pendings = []   # [b, st, p_exp, c_lo, c_hi, pT_sb] awaiting stage6
            for b in range(BPC):
                c_lo = cpool.tile([128, 512], F32, name=f"c_lo{b}", tag="c_lo")
                c_hi = cpool.tile([128, 512], F32, name=f"c_hi{b}", tag="c_hi")
                S4_sb = smalls.tile([1, NST], F32, name=f"S4_sb{b}", tag="S4_sb")
                S4_tiles[b] = S4_sb
                for st in range(NST):
                    hT_sb = ht_tiles.pop((b, st))
                    p_row = prowp.tile([1, ST], F32, name=f"p_row{b}_{st}",
                                       tag="p_row")
                    e_sbs = []
                    for ac in range(NAC):
                        e_ps = epool.tile([128, ST], F32, name=f"e_ps{b}_{st}_{ac}",
                                          tag="e_ps")
                        for dc in range(NDC):
                            nc.tensor.matmul(
                                e_ps,
                                lhsT=U_sb[:, dc, 128 * ac:128 * (ac + 1)],
                                rhs=hT_sb[:, dc, :, :],
                                start=(dc == 0), stop=(dc == NDC - 1))
                        if b == 0 and st == 0 and ac == 0:
                            emit_bias()
                        e_sb = esbp.tile([128, ST], BF16, name=f"e_sb{b}_{st}_{ac}",
                                         tag="e_sb")
                        nc.scalar.activation(e_sb, e_ps, AF.Tanh,
                                             bias=bias_sb[:, ac, b:b + 1])
                        e_sbs.append(e_sb)
                        if ac == 1 and pendings:
                            # pT matmuls of the previous supertile: their exp
                            # dep is long done; DVE copy overlaps mains ac1-3.
                            e = pendings[-1]
                            if e[5] is None:
                                e[5] = stage6a(*e[:3])
                        if ac == 2:
                            # rolling prefetch: load 2 supertiles ahead
                            glob = NST * b + st + 2
                            if glob < NST * BPC:
                                load_h(glob // NST, glob % NST)
                    # transpose the next supertile (PE + DVE copies); also
                    # gives the last tanh time before the v-dots need it.
                    glob = NST * b + st + 1
                    if glob < NST * BPC:
                        xbar_h(glob // NST, glob % NST)
                    # v-dots after all mains: their tanh deps are resolved by
                    # the time PE reaches them.
                    for ac in range(NAC):
                        nc.tensor.matmul(p_row, lhsT=v_sb[:, ac:ac + 1],
                                         rhs=e_sbs[ac],
                                         start=(ac == 0), stop=(ac == NAC - 1))

                    p_exp = smalls.tile([1, ST], BF16, name=f"p_exp{b}_{st}",
                                        tag="p_exp")
                    nc.scalar.activation(p_exp, p_row, AF.Exp,
                                         accum_out=S4_sb[:, st:st + 1])

                    if len(pendings) >= 1:
                        e = pendings.pop(0)
                        stage6b(e[0], e[1], e[5], e[3], e[4])
                        if e[1] == NST - 1:   # finished a batch
                            c_out_tiles.append(
                                (e[0], batch_epilogue(e[0], e[3], e[4],
                                                      S4_tiles[e[0]])))
                    pendings.append([b, st, p_exp, c_lo, c_hi, None])
            # drain remaining pendings
            for e in pendings:
                if e[5] is None:
                    e[5] = stage6a(*e[:3])
                stage6b(e[0], e[1], e[5], e[3], e[4])
                if e[1] == NST - 1:
                    c_out_tiles.append(
                        (e[0], batch_epilogue(e[0], e[3], e[4],
                                              S4_tiles[e[0]])))

            # ---- all output DMAs at the very end (single mode transition) --
            for pb, c_sb in c_out_tiles:
                nc.gpsimd.dma_start(out=c[pb:pb + 1, :], in_=c_sb)

    nc.finalize()
    return nc


_NC_CACHE = None


def kernel(s, h, W_a, U_a, v_a):
    global _NC_CACHE
    if _NC_CACHE is None:
        _NC_CACHE = build_nc()
    nc = _NC_CACHE
    s = np.ascontiguousarray(s, dtype=np.float32)
    h = np.ascontiguousarray(h, dtype=np.float32)
    W_a = np.ascontiguousarray(W_a, dtype=np.float32)
    U_a = np.ascontiguousarray(U_a, dtype=np.float32)
    v_a = np.ascontiguousarray(v_a, dtype=np.float32)
    in_maps = [
        {"s": s[i * BPC:(i + 1) * BPC], "h": h[i * BPC:(i + 1) * BPC],
         "W_a": W_a, "U_a": U_a, "v_a": v_a}
        for i in range(N_CORES)
    ]
    res = run_bass_kernel_spmd(nc, in_maps, core_ids=list(range(N_CORES)))
    return np.concatenate([res.results[i]["c"] for i in range(N_CORES)], axis=0)


# revision 27
# speedup vs baseline: 1.0428x; 1.0006x over previous
"""Trainium2 Bass kernel for additive (Bahdanau) attention.

    c[b] = softmax_t( v_a . tanh(s[b] @ W_a + h[b] @ U_a) ) @ h[b]

Shapes (hardcoded): s [32,1024] f32, h [32,2048,1024] f32,
W_a [1024,512], U_a [1024,512], v_a [512]  ->  c [32,1024] f32.

Sharding: data-parallel over batch; 8 NeuronCores x 4 batches each.
W_a/U_a/v_a replicated. No cross-core communication.

Key structural constraints learned from profiling:
  - copy-mode DMAs and xbar transpose DMAs serialize on HW (fully additive,
    measured), and the xbar runs at only ~150 GB/s -> do the transposes on
    the TensorEngine instead (is_transpose matmuls, bf16 PSUM out, DVE 2x
    copy-back) and keep the DMA pipe copy-only at HBM line rate.
  - The PE queue is in-order; any instruction waiting on a cross-engine dep
    stalls everything behind it -> defer the softmax/stage-6 tail by one
    supertile and issue v-dot matmuls only after all mains of a supertile.

Per-core pipeline, per 512-row "supertile" of h[b]:
  1. SWDGE DMA loads h f32->bf16 natural layout [128t, 4ts, 1024d].
  2. TensorE transposes 32 [128,128] chunks (identity moving operand) into
     bf16 PSUM banks; VectorE copies them to SBUF as [128 d_lo, dc, ts, t].
  3. TensorE: 32 bf16 matmuls (U_a chunks stationary) -> PSUM E [a_chunk, t].
  4. ScalarE: tanh(E + bias) with per-partition bias (W_a @ s), bf16 out.
  5. TensorE: v-dot (v chunk stationary, E moving) -> logit row [1, 512].
  6. ScalarE: exp (+ accumulated row sum S) -> p row (unnormalized softmax;
     e is bounded by |v|_1 so no max subtraction is needed).
  7. TensorE: transpose p chunks to [128,1] via K=1 matmul vs [1,1] ones.
  8. TensorE: c += p^T @ h_natural (PSUM-accumulated over the whole batch).
  9. batch end: c = c * (1/S) on VectorE; all output DMAs at kernel end.
"""

import numpy as np

import concourse.bacc as bacc
import concourse.tile as tile
import concourse.mybir as mybir
from concourse.bass_utils import run_bass_kernel_spmd

N_CORES = 8
B, T, DH, DS, A = 32, 2048, 1024, 1024, 512
BPC = B // N_CORES          # batches per core
ST = 512                    # supertile rows (t)
NST = T // ST               # supertiles per batch
NTS = ST // 128             # 128-row chunks per supertile
NDC = DH // 128             # d chunks
NAC = A // 128              # a chunks
NTCH = T // 128             # 128-row chunks per batch

F32 = mybir.dt.float32
BF16 = mybir.dt.bfloat16
AF = mybir.ActivationFunctionType


def build_nc():
    nc = bacc.Bacc("TRN2", target_bir_lowering=False, debug=False,
                   num_devices=N_CORES)
    s = nc.dram_tensor("s", [BPC, DS], F32, kind="ExternalInput").ap()
    h = nc.dram_tensor("h", [BPC, T, DH], F32, kind="ExternalInput").ap()
    W_a = nc.dram_tensor("W_a", [DS, A], F32, kind="ExternalInput").ap()
    U_a = nc.dram_tensor("U_a", [DH, A], F32, kind="ExternalInput").ap()
    v_a = nc.dram_tensor("v_a", [A], F32, kind="ExternalInput").ap()
    c = nc.dram_tensor("c", [BPC, DH], F32, kind="ExternalOutput").ap()

    with tile.TileContext(nc) as tc:
        with (
            tc.tile_pool(name="const", bufs=1) as const,
            tc.tile_pool(name="hpool", bufs=8) as hpool,
            tc.tile_pool(name="htpool", bufs=4) as htpool,
            tc.tile_pool(name="esbp", bufs=6) as esbp,
            tc.tile_pool(name="smalls", bufs=4) as smalls,
            tc.tile_pool(name="cresp", bufs=4) as cresp,
            tc.tile_pool(name="epool", bufs=2, space="PSUM") as epool,
            tc.tile_pool(name="prowp", bufs=1, space="PSUM") as prowp,
            tc.tile_pool(name="ptpp", bufs=1, space="PSUM") as ptpp,
            tc.tile_pool(name="cpool", bufs=1, space="PSUM") as cpool,
            tc.tile_pool(name="tpsp", bufs=2, space="PSUM") as tpsp,
        ):
            h_tiles = {}
            ht_tiles = {}

            def load_h(b, st):
                t = hpool.tile([128, NTS, DH], BF16, name=f"h_sb{b}_{st}",
                               tag="h_sb")
                nc.gpsimd.dma_start(
                    out=t,
                    in_=h[b, ST * st:ST * (st + 1), :]
                    .rearrange("(ts p) d -> p ts d", p=128))
                h_tiles[(b, st)] = t

            def xbar_h(b, st):
                # PE-based transpose: 32 [128,128] chunks -> 4 bf16 PSUM banks
                # (2 d-chunks each) -> DVE 2x copy to SBUF.
                # hT layout: [128 d_lo, NDC, NTS, 128 t].
                h_sb = h_tiles[(b, st)]
                ht = htpool.tile([128, NDC, NTS, 128], BF16,
                                 name=f"hT_sb{b}_{st}", tag="hT_sb")
                for dcp in range(NDC // 2):
                    tps = tpsp.tile([128, 1024], BF16,
                                    name=f"tps{b}_{st}_{dcp}", tag="tps")
                    for dch in range(2):
                        dc = 2 * dcp + dch
                        for ts in range(NTS):
                            nc.tensor.transpose(
                                tps[:, dch * 512 + ts * 128:
                                    dch * 512 + ts * 128 + 128],
                                h_sb[:, ts, 128 * dc:128 * (dc + 1)],
                                ident)
                    nc.vector.tensor_copy(
                        ht[:, 2 * dcp, :, :], tps[:, 0:512])
                    nc.vector.tensor_copy(
                        ht[:, 2 * dcp + 1, :, :], tps[:, 512:1024])
                ht_tiles[(b, st)] = ht

            from concourse.masks import make_identity
            ident = const.tile([128, 128], BF16, name="ident")
            make_identity(nc, ident)

            # -- startup: first load split into quarters so the first PE
            # transposes unblock per-chunk; then its transpose.
            t0 = hpool.tile([128, NTS, DH], BF16, name="h_sb0_0", tag="h_sb")
            for ts in range(NTS):
                nc.gpsimd.dma_start(
                    out=t0[:, ts],
                    in_=h[0, 128 * ts:128 * (ts + 1), :]
                    .rearrange("p d -> p d"))
            h_tiles[(0, 0)] = t0
            xbar_h(0, 0)

            # ---- constants (copy-mode phase) ----
            U_sb = const.tile([128, NDC, A], BF16)
            nc.gpsimd.dma_start(out=U_sb, in_=U_a.rearrange("(dc p) a -> p dc a", p=128))
            load_h(0, 1)
            W_sb = const.tile([128, NDC, A], F32)
            nc.gpsimd.dma_start(out=W_sb, in_=W_a.rearrange("(dc p) a -> p dc a", p=128))
            sT_sb = const.tile([128, NDC, BPC], F32)
            for dc in range(NDC):
                nc.gpsimd.dma_start(
                    out=sT_sb[:, dc, :],
                    in_=s[:, 128 * dc:128 * (dc + 1)].rearrange("b p -> p b"))
            v_sb = const.tile([128, NAC], BF16)
            nc.gpsimd.dma_start(out=v_sb, in_=v_a.rearrange("(ac p) -> p ac", p=128))
            one1 = const.tile([1, 1], BF16)
            nc.vector.memset(one1, 1.0)
            load_h(0, 2)

            # bias[a, b] = (W_a^T s[b])[a]  stored [128 a_lo, NAC, BPC] f32
            bias_sb = const.tile([128, NAC, BPC], F32)

            def emit_bias():
                for ac in range(NAC):
                    ws_ps = epool.tile([128, BPC], F32, name=f"ws_ps{ac}",
                                       tag="e_ps")
                    for dc in range(NDC):
                        nc.tensor.matmul(ws_ps,
                                         lhsT=W_sb[:, dc, 128 * ac:128 * (ac + 1)],
                                         rhs=sT_sb[:, dc, :],
                                         start=(dc == 0), stop=(dc == NDC - 1))
                    nc.vector.tensor_copy(bias_sb[:, ac, :], ws_ps)

            def stage6a(b, st, p_exp):
                # p-row -> column transpose matmuls + copy to SBUF
                pT_ps = ptpp.tile([128, NTS], F32, name=f"pT_ps{b}_{st}",
                                  tag="pT_ps")
                for ts in range(NTS):
                    nc.tensor.matmul(pT_ps[:, ts:ts + 1],
                                     lhsT=p_exp[:, 128 * ts:128 * (ts + 1)],
                                     rhs=one1, start=True, stop=True,
                                     skip_group_check=True)
                pT_sb = smalls.tile([128, NTS], BF16, name=f"pT_sb{b}_{st}",
                                    tag="pT_sb")
                nc.vector.tensor_copy(pT_sb, pT_ps)
                return pT_sb

            def stage6b(b, st, pT_sb, c_lo, c_hi):
                # c matmuls are M=1: pack the 4 t-chunks into 4 column groups
                # (tile_position) so they run concurrently; partial sums land
                # on partitions 0/32/64/96 and are combined at batch end.
                h_sb = h_tiles.pop((b, st))
                first, last = st == 0, st == NST - 1
                for ts in range(NTS):
                    nc.tensor.matmul(c_lo[32 * ts:32 * ts + 1, :],
                                     lhsT=pT_sb[:, ts:ts + 1],
                                     rhs=h_sb[:, ts, 0:512],
                                     start=first, stop=last,
                                     tile_position=(0, 32 * ts),
                                     skip_group_check=True)
                    nc.tensor.matmul(c_hi[32 * ts:32 * ts + 1, :],
                                     lhsT=pT_sb[:, ts:ts + 1],
                                     rhs=h_sb[:, ts, 512:DH],
                                     start=first, stop=last,
                                     tile_position=(0, 32 * ts),
                                     skip_group_check=True)

            def batch_epilogue(b, c_lo, c_hi, S4_sb):
                S_sb = smalls.tile([1, 1], F32, name=f"S_sb{b}", tag="S_sb")
                nc.vector.reduce_sum(S_sb, S4_sb, axis=mybir.AxisListType.X)
                rS = smalls.tile([1, 1], F32, name=f"rS{b}", tag="rS")
                nc.vector.reciprocal(rS, S_sb)
                c4_sb = cresp.tile([128, 2, 512], F32, name=f"c4_sb{b}",
                                   tag="c4_sb", bufs=2)
                nc.vector.tensor_copy(c4_sb[:, 0, :], c_lo)
                nc.vector.tensor_copy(c4_sb[:, 1, :], c_hi)
                acc = cresp.tile([1, DH], F32, name=f"acc{b}", tag=f"acc{b}",
                                 bufs=1)
                # fold rows 0/32/64/96 with chained accumulate-add DMAs
                acc2d = acc.rearrange("o (k d) -> o k d", k=2)
                nc.gpsimd.dma_start(out=acc2d, in_=c4_sb[0:1])
                for j in range(1, NTS):
                    nc.gpsimd.dma_start(out=acc2d, in_=c4_sb[32 * j:32 * j + 1],
                                        accum_op=mybir.AluOpType.add)
                c_sb = cresp.tile([1, DH], F32, name=f"c_sb{b}", tag=f"c_sb{b}",
                                  bufs=1)
                nc.vector.tensor_scalar_mul(c_sb, acc, rS)
                return c_sb

            # ---- main loop ----
            c_out_tiles = []
            S4_tiles = {}
            pendings = []   # [b, st, p_exp, c_lo, c_hi, pT_sb] awaiting stage6
            for b in range(BPC):
                c_lo = cpool.tile([128, 512], F32, name=f"c_lo{b}", tag="c_lo")
                c_hi = cpool.tile([128, 512], F32, name=f"c_hi{b}", tag="c_hi")
                S4_sb = smalls.tile([1, NST], F32, name=f"S4_sb{b}", tag="S4_sb")
                S4_tiles[b] = S4_sb
                for st in range(NST):
                    hT_sb = ht_tiles.pop((b, st))
                    p_row = prowp.tile([1, ST], F32, name=f"p_row{b}_{st}",
                                       tag="p_row")
                    e_sbs = []
                    for ac in range(NAC):
                        e_ps = epool.tile([128, ST], F32, name=f"e_ps{b}_{st}_{ac}",
                                          tag="e_ps")
                        for dc in range(NDC):
                            nc.tensor.matmul(
                                e_ps,
                                lhsT=U_sb[:, dc, 128 * ac:128 * (ac + 1)],
                                rhs=hT_sb[:, dc, :, :],
                                start=(dc == 0), stop=(dc == NDC - 1))
                        if b == 0 and st == 0 and ac == 0:
                            emit_bias()
                        e_sb = esbp.tile([128, ST], BF16, name=f"e_sb{b}_{st}_{ac}",
                                         tag="e_sb")
                        nc.scalar.activation(e_sb, e_ps, AF.Tanh,
                                             bias=bias_sb[:, ac, b:b + 1])
                        e_sbs.append(e_sb)
                        if ac == 1 and pendings:
                            # pT matmuls of the previous supertile: their exp
                            # dep is long done; DVE copy overlaps mains ac1-3.
                            e = pendings[-1]
                            if e[5] is None:
                                e[5] = stage6a(*e[:3])
                        if ac == 2:
                            # rolling prefetch: load 2 supertiles ahead
                            glob = NST * b + st + 2
                            if glob < NST * BPC:
                                load_h(glob // NST, glob % NST)
                    # transpose the next supertile (PE + DVE copies); also
                    # gives the last tanh time before the v-dots need it.
                    glob = NST * b + st + 1
                    if glob < NST * BPC:
                        xbar_h(glob // NST, glob % NST)
                    # v-dots after all mains: their tanh deps are resolved by
                    # the time PE reaches them.
                    for ac in range(NAC):
                        nc.tensor.matmul(p_row, lhsT=v_sb[:, ac:ac + 1],
                                         rhs=e_sbs[ac],
                                         start=(ac == 0), stop=(ac == NAC - 1))

                    p_exp = smalls.tile([1, ST], BF16, name=f"p_exp{b}_{st}",
                                        tag="p_exp")
                    nc.scalar.activation(p_exp, p_row, AF.Exp,
                                         accum_out=S4_sb[:, st:st + 1])

                    if len(pendings) >= 1:
                        e = pendings.pop(0)
                        stage6b(e[0], e[1], e[5], e[3], e[4])
                        if e[1] == NST - 1:   # finished a batch
                            c_out_tiles.append(
                                (e[0], batch_epilogue(e[0], e[3], e[4],
                                                      S4_tiles[e[0]])))
                    pendings.append([b, st, p_exp, c_lo, c_hi, None])
            # drain remaining pendings
            for e in pendings:
                if e[5] is None:
                    e[5] = stage6a(*e[:3])
                stage6b(e[0], e[1], e[5], e[3], e[4])
                if e[1] == NST - 1:
                    c_out_tiles.append(
                        (e[0], batch_epilogue(e[0], e[3], e[4],
                                              S4_tiles[e[0]])))

            # ---- all output DMAs at the very end (single mode transition) --
            for pb, c_sb in c_out_tiles:
                nc.gpsimd.dma_start(out=c[pb:pb + 1, :], in_=c_sb)

    nc.finalize()
    return nc


_NC_CACHE = None


def kernel(s, h, W_a, U_a, v_a):
    global _NC_CACHE
    if _NC_CACHE is None:
        _NC_CACHE = build_nc()
    nc = _NC_CACHE
    s = np.ascontiguousarray(s, dtype=np.float32)
    h = np.ascontiguousarray(h, dtype=np.float32)
    W_a = np.ascontiguousarray(W_a, dtype=np.float32)
    U_a = np.ascontiguousarray(U_a, dtype=np.float32)
    v_a = np.ascontiguousarray(v_a, dtype=np.float32)
    in_maps = [
        {"s": s[i * BPC:(i + 1) * BPC], "h": h[i * BPC:(i + 1) * BPC],
         "W_a": W_a, "U_a": U_a, "v_a": v_a}
        for i in range(N_CORES)
    ]
    res = run_bass_kernel_spmd(nc, in_maps, core_ids=list(range(N_CORES)))
    return np.concatenate([res.results[i]["c"] for i in range(N_CORES)], axis=0)


# revision 28
# speedup vs baseline: 1.0493x; 1.0062x over previous
"""Trainium2 Bass kernel for additive (Bahdanau) attention.

    c[b] = softmax_t( v_a . tanh(s[b] @ W_a + h[b] @ U_a) ) @ h[b]

Shapes (hardcoded): s [32,1024] f32, h [32,2048,1024] f32,
W_a [1024,512], U_a [1024,512], v_a [512]  ->  c [32,1024] f32.

Sharding: data-parallel over batch; 8 NeuronCores x 4 batches each.
W_a/U_a/v_a replicated. No cross-core communication.

Key structural constraints learned from profiling:
  - copy-mode DMAs and xbar transpose DMAs serialize on HW (fully additive,
    measured), and the xbar runs at only ~150 GB/s -> do the transposes on
    the TensorEngine instead (is_transpose matmuls, bf16 PSUM out, DVE 2x
    copy-back) and keep the DMA pipe copy-only at HBM line rate.
  - The PE queue is in-order; any instruction waiting on a cross-engine dep
    stalls everything behind it -> defer the softmax/stage-6 tail by one
    supertile and issue v-dot matmuls only after all mains of a supertile.

Per-core pipeline, per 512-row "supertile" of h[b]:
  1. SWDGE DMA loads h f32->bf16 natural layout [128t, 4ts, 1024d].
  2. TensorE transposes 32 [128,128] chunks (identity moving operand) into
     bf16 PSUM banks; VectorE copies them to SBUF as [128 d_lo, dc, ts, t].
  3. TensorE: 32 bf16 matmuls (U_a chunks stationary) -> PSUM E [a_chunk, t].
  4. ScalarE: tanh(E + bias) with per-partition bias (W_a @ s), bf16 out.
  5. TensorE: v-dot (v chunk stationary, E moving) -> logit row [1, 512].
  6. ScalarE: exp (+ accumulated row sum S) -> p row (unnormalized softmax;
     e is bounded by |v|_1 so no max subtraction is needed).
  7. TensorE: transpose p chunks to [128,1] via K=1 matmul vs [1,1] ones.
  8. TensorE: c += p^T @ h_natural (PSUM-accumulated over the whole batch).
  9. batch end: c = c * (1/S) on VectorE; all output DMAs at kernel end.
"""

import numpy as np

import concourse.bacc as bacc
import concourse.tile as tile
import concourse.mybir as mybir
from concourse.bass_utils import run_bass_kernel_spmd

N_CORES = 8
B, T, DH, DS, A = 32, 2048, 1024, 1024, 512
BPC = B // N_CORES          # batches per core
ST = 512                    # supertile rows (t)
NST = T // ST               # supertiles per batch
NTS = ST // 128             # 128-row chunks per supertile
NDC = DH // 128             # d chunks
NAC = A // 128              # a chunks
NTCH = T // 128             # 128-row chunks per batch

F32 = mybir.dt.float32
BF16 = mybir.dt.bfloat16
AF = mybir.ActivationFunctionType


def build_nc():
    nc = bacc.Bacc("TRN2", target_bir_lowering=False, debug=False,
                   num_devices=N_CORES)
    s = nc.dram_tensor("s", [BPC, DS], F32, kind="ExternalInput").ap()
    h = nc.dram_tensor("h", [BPC, T, DH], F32, kind="ExternalInput").ap()
    W_a = nc.dram_tensor("W_a", [DS, A], F32, kind="ExternalInput").ap()
    U_a = nc.dram_tensor("U_a", [DH, A], F32, kind="ExternalInput").ap()
    v_a = nc.dram_tensor("v_a", [A], F32, kind="ExternalInput").ap()
    c = nc.dram_tensor("c", [BPC, DH], F32, kind="ExternalOutput").ap()

    with tile.TileContext(nc) as tc:
        with (
            tc.tile_pool(name="const", bufs=1) as const,
            tc.tile_pool(name="hpool", bufs=8) as hpool,
            tc.tile_pool(name="htpool", bufs=4) as htpool,
            tc.tile_pool(name="esbp", bufs=6) as esbp,
            tc.tile_pool(name="smalls", bufs=4) as smalls,
            tc.tile_pool(name="cresp", bufs=4) as cresp,
            tc.tile_pool(name="epool", bufs=3, space="PSUM") as epool,
            tc.tile_pool(name="ppool", bufs=1, space="PSUM") as ppool,
            tc.tile_pool(name="cpool", bufs=1, space="PSUM") as cpool,
            tc.tile_pool(name="tpsp", bufs=2, space="PSUM") as tpsp,
        ):
            h_tiles = {}
            ht_tiles = {}

            def load_h(b, st):
                t = hpool.tile([128, NTS, DH], BF16, name=f"h_sb{b}_{st}",
                               tag="h_sb")
                nc.gpsimd.dma_start(
                    out=t,
                    in_=h[b, ST * st:ST * (st + 1), :]
                    .rearrange("(ts p) d -> p ts d", p=128))
                h_tiles[(b, st)] = t

            def xbar_h(b, st):
                # PE-based transpose: 32 [128,128] chunks -> 4 bf16 PSUM banks
                # (2 d-chunks each) -> DVE 2x copy to SBUF.
                # hT layout: [128 d_lo, NDC, NTS, 128 t].
                h_sb = h_tiles[(b, st)]
                ht = htpool.tile([128, NDC, NTS, 128], BF16,
                                 name=f"hT_sb{b}_{st}", tag="hT_sb")
                for dcp in range(NDC // 2):
                    tps = tpsp.tile([128, 1024], BF16,
                                    name=f"tps{b}_{st}_{dcp}", tag="tps")
                    for dch in range(2):
                        dc = 2 * dcp + dch
                        for ts in range(NTS):
                            nc.tensor.transpose(
                                tps[:, dch * 512 + ts * 128:
                                    dch * 512 + ts * 128 + 128],
                                h_sb[:, ts, 128 * dc:128 * (dc + 1)],
                                ident)
                    nc.vector.tensor_copy(
                        ht[:, 2 * dcp, :, :], tps[:, 0:512])
                    nc.vector.tensor_copy(
                        ht[:, 2 * dcp + 1, :, :], tps[:, 512:1024])
                ht_tiles[(b, st)] = ht

            from concourse.masks import make_identity
            ident = const.tile([128, 128], BF16, name="ident")
            make_identity(nc, ident)

            # -- startup: first load split into quarters so the first PE
            # transposes unblock per-chunk; then its transpose.
            t0 = hpool.tile([128, NTS, DH], BF16, name="h_sb0_0", tag="h_sb")
            for ts in range(NTS):
                nc.gpsimd.dma_start(
                    out=t0[:, ts],
                    in_=h[0, 128 * ts:128 * (ts + 1), :]
                    .rearrange("p d -> p d"))
            h_tiles[(0, 0)] = t0
            xbar_h(0, 0)

            # ---- constants (copy-mode phase) ----
            U_sb = const.tile([128, NDC, A], BF16)
            nc.gpsimd.dma_start(out=U_sb, in_=U_a.rearrange("(dc p) a -> p dc a", p=128))
            load_h(0, 1)
            W_sb = const.tile([128, NDC, A], F32)
            nc.gpsimd.dma_start(out=W_sb, in_=W_a.rearrange("(dc p) a -> p dc a", p=128))
            sT_sb = const.tile([128, NDC, BPC], F32)
            for dc in range(NDC):
                nc.gpsimd.dma_start(
                    out=sT_sb[:, dc, :],
                    in_=s[:, 128 * dc:128 * (dc + 1)].rearrange("b p -> p b"))
            v_sb = const.tile([128, NAC], BF16)
            nc.gpsimd.dma_start(out=v_sb, in_=v_a.rearrange("(ac p) -> p ac", p=128))
            one1 = const.tile([1, 1], BF16)
            nc.vector.memset(one1, 1.0)
            load_h(0, 2)

            # bias[a, b] = (W_a^T s[b])[a]  stored [128 a_lo, NAC, BPC] f32
            bias_sb = const.tile([128, NAC, BPC], F32)

            def emit_bias():
                for ac in range(NAC):
                    ws_ps = epool.tile([128, BPC], F32, name=f"ws_ps{ac}",
                                       tag="e_ps")
                    for dc in range(NDC):
                        nc.tensor.matmul(ws_ps,
                                         lhsT=W_sb[:, dc, 128 * ac:128 * (ac + 1)],
                                         rhs=sT_sb[:, dc, :],
                                         start=(dc == 0), stop=(dc == NDC - 1))
                    nc.vector.tensor_copy(bias_sb[:, ac, :], ws_ps)

            def stage6a(b, st, p_exp):
                # p-row -> column transpose matmuls + copy to SBUF
                pT_ps = ppool.tile([128, NTS], F32, name=f"pT_ps{b}_{st}",
                                  tag="pp", padded_shape=[128, 512])
                for ts in range(NTS):
                    nc.tensor.matmul(pT_ps[:, ts:ts + 1],
                                     lhsT=p_exp[:, 128 * ts:128 * (ts + 1)],
                                     rhs=one1, start=True, stop=True,
                                     skip_group_check=True)
                pT_sb = smalls.tile([128, NTS], BF16, name=f"pT_sb{b}_{st}",
                                    tag="pT_sb")
                nc.vector.tensor_copy(pT_sb, pT_ps)
                return pT_sb

            def stage6b(b, st, pT_sb, c_lo, c_hi):
                # c matmuls are M=1: pack the 4 t-chunks into 4 column groups
                # (tile_position) so they run concurrently; partial sums land
                # on partitions 0/32/64/96 and are combined at batch end.
                h_sb = h_tiles.pop((b, st))
                first, last = st == 0, st == NST - 1
                for ts in range(NTS):
                    nc.tensor.matmul(c_lo[32 * ts:32 * ts + 1, :],
                                     lhsT=pT_sb[:, ts:ts + 1],
                                     rhs=h_sb[:, ts, 0:512],
                                     start=first, stop=last,
                                     tile_position=(0, 32 * ts),
                                     skip_group_check=True)
                    nc.tensor.matmul(c_hi[32 * ts:32 * ts + 1, :],
                                     lhsT=pT_sb[:, ts:ts + 1],
                                     rhs=h_sb[:, ts, 512:DH],
                                     start=first, stop=last,
                                     tile_position=(0, 32 * ts),
                                     skip_group_check=True)

            def batch_epilogue(b, c_lo, c_hi, S4_sb):
                S_sb = smalls.tile([1, 1], F32, name=f"S_sb{b}", tag="S_sb")
                nc.vector.reduce_sum(S_sb, S4_sb, axis=mybir.AxisListType.X)
                rS = smalls.tile([1, 1], F32, name=f"rS{b}", tag="rS")
                nc.vector.reciprocal(rS, S_sb)
                c4_sb = cresp.tile([128, 2, 512], F32, name=f"c4_sb{b}",
                                   tag="c4_sb", bufs=2)
                nc.vector.tensor_copy(c4_sb[:, 0, :], c_lo)
                nc.vector.tensor_copy(c4_sb[:, 1, :], c_hi)
                acc = cresp.tile([1, DH], F32, name=f"acc{b}", tag=f"acc{b}",
                                 bufs=1)
                # fold rows 0/32/64/96 with chained accumulate-add DMAs
                acc2d = acc.rearrange("o (k d) -> o k d", k=2)
                nc.gpsimd.dma_start(out=acc2d, in_=c4_sb[0:1])
                for j in range(1, NTS):
                    nc.gpsimd.dma_start(out=acc2d, in_=c4_sb[32 * j:32 * j + 1],
                                        accum_op=mybir.AluOpType.add)
                c_sb = cresp.tile([1, DH], F32, name=f"c_sb{b}", tag=f"c_sb{b}",
                                  bufs=1)
                nc.vector.tensor_scalar_mul(c_sb, acc, rS)
                return c_sb

            # ---- main loop ----
            c_out_tiles = []
            S4_tiles = {}
            pendings = []   # [b, st, p_exp, c_lo, c_hi, pT_sb] awaiting stage6
            for b in range(BPC):
                c_lo = cpool.tile([128, 512], F32, name=f"c_lo{b}", tag="c_lo")
                c_hi = cpool.tile([128, 512], F32, name=f"c_hi{b}", tag="c_hi")
                S4_sb = smalls.tile([1, NST], F32, name=f"S4_sb{b}", tag="S4_sb")
                S4_tiles[b] = S4_sb
                for st in range(NST):
                    hT_sb = ht_tiles.pop((b, st))
                    p_row = None
                    e_sbs = []
                    for ac in range(NAC):
                        e_ps = epool.tile([128, ST], F32, name=f"e_ps{b}_{st}_{ac}",
                                          tag="e_ps")
                        for dc in range(NDC):
                            nc.tensor.matmul(
                                e_ps,
                                lhsT=U_sb[:, dc, 128 * ac:128 * (ac + 1)],
                                rhs=hT_sb[:, dc, :, :],
                                start=(dc == 0), stop=(dc == NDC - 1))
                        if b == 0 and st == 0 and ac == 0:
                            emit_bias()
                        e_sb = esbp.tile([128, ST], BF16, name=f"e_sb{b}_{st}_{ac}",
                                         tag="e_sb")
                        nc.scalar.activation(e_sb, e_ps, AF.Tanh,
                                             bias=bias_sb[:, ac, b:b + 1])
                        e_sbs.append(e_sb)
                        if ac == 1 and pendings:
                            # pT matmuls of the previous supertile: their exp
                            # dep is long done; DVE copy overlaps mains ac1-3.
                            e = pendings[-1]
                            if e[5] is None:
                                e[5] = stage6a(*e[:3])
                        if ac == 2:
                            # rolling prefetch: load 2 supertiles ahead
                            glob = NST * b + st + 2
                            if glob < NST * BPC:
                                load_h(glob // NST, glob % NST)
                    # transpose the next supertile (PE + DVE copies); also
                    # gives the last tanh time before the v-dots need it.
                    glob = NST * b + st + 1
                    if glob < NST * BPC:
                        xbar_h(glob // NST, glob % NST)
                    # v-dots after all mains: their tanh deps are resolved by
                    # the time PE reaches them. p_row allocated late so the
                    # shared ppool slot ring-orders pT(st-1) -> p_row(st).
                    p_row = ppool.tile([1, ST], F32, name=f"p_row{b}_{st}",
                                       tag="pp", padded_shape=[128, 512])
                    for ac in range(NAC):
                        nc.tensor.matmul(p_row, lhsT=v_sb[:, ac:ac + 1],
                                         rhs=e_sbs[ac],
                                         start=(ac == 0), stop=(ac == NAC - 1))

                    p_exp = smalls.tile([1, ST], BF16, name=f"p_exp{b}_{st}",
                                        tag="p_exp")
                    nc.scalar.activation(p_exp, p_row, AF.Exp,
                                         accum_out=S4_sb[:, st:st + 1])

                    if len(pendings) >= 1:
                        e = pendings.pop(0)
                        stage6b(e[0], e[1], e[5], e[3], e[4])
                        if e[1] == NST - 1:   # finished a batch
                            c_out_tiles.append(
                                (e[0], batch_epilogue(e[0], e[3], e[4],
                                                      S4_tiles[e[0]])))
                    pendings.append([b, st, p_exp, c_lo, c_hi, None])
            # drain remaining pendings
            for e in pendings:
                if e[5] is None:
                    e[5] = stage6a(*e[:3])
                stage6b(e[0], e[1], e[5], e[3], e[4])
                if e[1] == NST - 1:
                    c_out_tiles.append(
                        (e[0], batch_epilogue(e[0], e[3], e[4],
                                              S4_tiles[e[0]])))

            # ---- all output DMAs at the very end (single mode transition) --
            for pb, c_sb in c_out_tiles:
                nc.gpsimd.dma_start(out=c[pb:pb + 1, :], in_=c_sb)

    nc.finalize()
    return nc


_NC_CACHE = None


def kernel(s, h, W_a, U_a, v_a):
    global _NC_CACHE
    if _NC_CACHE is None:
        _NC_CACHE = build_nc()
    nc = _NC_CACHE
    s = np.ascontiguousarray(s, dtype=np.float32)
    h = np.ascontiguousarray(h, dtype=np.float32)
    W_a = np.ascontiguousarray(W_a, dtype=np.float32)
    U_a = np.ascontiguousarray(U_a, dtype=np.float32)
    v_a = np.ascontiguousarray(v_a, dtype=np.float32)
    in_maps = [
        {"s": s[i * BPC:(i + 1) * BPC], "h": h[i * BPC:(i + 1) * BPC],
         "W_a": W_a, "U_a": U_a, "v_a": v_a}
        for i in range(N_CORES)
    ]
    res = run_bass_kernel_spmd(nc, in_maps, core_ids=list(range(N_CORES)))
    return np.concatenate([res.results[i]["c"] for i in range(N_CORES)], axis=0)
